# revision 4
# baseline (speedup 1.0000x reference)
"""Trainium2 Bass kernel for cascaded double cross-attention.

Reference computation (B=2, N=2048, C=1024, H=16, D=64):
    q = heads(x @ Wq.T); A = heads(x2 @ Wa.T); k, v = heads(x @ Wkv.T)
    ATT_q = softmax(q @ A^T * s);  ATT_k = softmax(A @ k^T * s)
    out = ATT_q @ (ATT_k @ v)

Sharding: 8 cores, core i handles batch b=i//4 and 4 heads g=i%4 (heads
4g..4g+3).  Host pre-transposes/casts inputs so the device kernel needs no
transposes of x: each core receives xT/x2T ([C, N] bf16) and per-head-group
weight slices WqT/WaT/WkT/WvT ([C, 256] bf16).

Device dataflow: heads processed in PAIRS (head 2p in partitions 0:64,
head 2p+1 in 64:128 of the qt/at/kt pair tiles).  The two heads' K=64
score matmuls are issued back-to-back from partition bases 0 and 64, so
they run CONCURRENTLY in disjoint row-groups of the PE array (and each
LDWEIGHTS overlaps the other group's in-flight matmul).

Per phase-pair (A1 = exp(k.A)@[v|1], A2 = exp(A.q)@[tmp|1]), two m-passes
of 1024 columns each so PSUM fits:
  score pool: 4 x [128,512] f32  (4 banks)
  acc per head: [128,1024] f32   (2 banks each; 8 blocks of 65 at offsets
                                  0,65,..,390 and 512; start=True on the
                                  first block per bank zeroes the bank)
exp is split between the scalar engine (exact table exp) and the vector
engine (Schraudolph: int16(x*23.083+16250.5) bitcast to bf16 -- one
tensor_scalar op; rel-err ~1.5% rms on P, negligible after softmax
averaging).  Row sums accumulate as the 65th AV column; per-pass tails
on DVE do reciprocal + normalize into [tmp|1] (A1) or output staging
(A2).
"""

import sys

if "/opt/trn_rl_repo" not in sys.path:
    sys.path.insert(0, "/opt/trn_rl_repo")

import numpy as np
import ml_dtypes

import concourse.bass as bass
import concourse.tile as tile
from concourse import bacc, mybir
from concourse.bass_utils import run_bass_kernel_spmd

BF16 = ml_dtypes.bfloat16
N_CORES = 8
N, C, H, D = 2048, 1024, 16, 64
HPC = 4  # heads per core
DHC = HPC * D  # 256 output cols per core
CCH = C // 128  # 8 contraction chunks
NB = N // 128  # 16 token blocks
SCALE = float(D) ** -0.5
F32 = mybir.dt.float32
BF = mybir.dt.bfloat16
I16 = mybir.dt.int16
EXP = mybir.ActivationFunctionType.Exp
MULT = mybir.AluOpType.mult
ADD = mybir.AluOpType.add

# Schraudolph exp constants for bf16 bitcast: int16(x*K + B) ~ exp(x*SCALE)
EXP_K = SCALE * 1.4426950408889634 * 128.0
EXP_B = 16256.0 - 5.5  # (127 - c)*128, c ~ 0.043 (round-to-nearest)

LAG = 3
ACT_OF_16 = 8  # exp tiles per 16 sent to the scalar engine (rest DVE)

_CACHE = {}


def _build_program(nreps=1):
    nc = bacc.Bacc("TRN2", target_bir_lowering=False, debug=False,
                   num_devices=N_CORES)

    xt_d = nc.dram_tensor("xt", [C, N], BF, kind="ExternalInput").ap()
    x2t_d = nc.dram_tensor("x2t", [C, N], BF, kind="ExternalInput").ap()
    wq_d = nc.dram_tensor("wq", [C, DHC], BF, kind="ExternalInput").ap()
    wa_d = nc.dram_tensor("wa", [C, DHC], BF, kind="ExternalInput").ap()
    wk_d = nc.dram_tensor("wk", [C, DHC], BF, kind="ExternalInput").ap()
    wv_d = nc.dram_tensor("wv", [C, DHC], BF, kind="ExternalInput").ap()
    out_d = nc.dram_tensor("out", [N, DHC], F32, kind="ExternalOutput").ap()

    with tile.TileContext(nc) as tc:
        for _ in range(nreps):
            _emit(tc, nc, xt_d, x2t_d, wq_d, wa_d, wk_d, wv_d, out_d)
    nc.compile()
    return nc


def _emit(tc, nc, xt_d, x2t_d, wq_d, wa_d, wk_d, wv_d, out_d):
    from contextlib import ExitStack

    ctx = ExitStack()
    with ctx:
        singles = ctx.enter_context(tc.tile_pool(name="singles", bufs=1))
        ppool = ctx.enter_context(tc.tile_pool(name="ptiles", bufs=10))
        tmpo_pool = ctx.enter_context(tc.tile_pool(name="tmpones", bufs=2))
        recp = ctx.enter_context(tc.tile_pool(name="recp", bufs=4))
        spool = ctx.enter_context(
            tc.tile_pool(name="spsum", bufs=4, space="PSUM"))
        apool = ctx.enter_context(
            tc.tile_pool(name="apsum", bufs=1, space="PSUM"))

        # ---- constants / persistent inputs ----
        xt_sb = singles.tile([128, CCH, N], BF, tag="xt")
        x2t_sb = singles.tile([128, CCH, N], BF, tag="x2t")
        w_sb = {}
        for name in ("wq", "wa", "wk", "wv"):
            w_sb[name] = singles.tile([128, CCH, DHC], BF, tag=name, name=name)
        nc.sync.dma_start(out=w_sb["wk"][:],
                          in_=wk_d.rearrange("(c p) d -> p c d", p=128))
        nc.scalar.dma_start(out=w_sb["wa"][:],
                            in_=wa_d.rearrange("(c p) d -> p c d", p=128))
        nc.gpsimd.dma_start(out=w_sb["wv"][:],
                            in_=wv_d.rearrange("(c p) d -> p c d", p=128))
        nc.gpsimd.dma_start(out=w_sb["wq"][:],
                            in_=wq_d.rearrange("(c p) d -> p c d", p=128))
        xt_r = xt_d.rearrange("(c p) n -> p c n", p=128)
        x2t_r = x2t_d.rearrange("(c p) n -> p c n", p=128)
        for q in range(4):
            ncol = slice(q * 512, (q + 1) * 512)
            nc.sync.dma_start(out=xt_sb[:, :, ncol], in_=xt_r[:, :, ncol])
            nc.scalar.dma_start(out=x2t_sb[:, :, ncol], in_=x2t_r[:, :, ncol])

        # per-pair transposed activations [128, N] bf16: head 2p in
        # partitions 0:64, head 2p+1 in partitions 64:128.
        qt_p = [singles.tile([128, N], BF, tag=f"qt{p}", name=f"qt{p}")
                for p in range(2)]
        at_p = [singles.tile([128, N], BF, tag=f"at{p}", name=f"at{p}")
                for p in range(2)]
        kt_p = [singles.tile([128, N], BF, tag=f"kt{p}", name=f"kt{p}")
                for p in range(2)]

        # staged full output [p, block, head, d] -> one contiguous out DMA
        ot_all = singles.tile([128, NB, HPC, D], F32, tag="ot_all")

        # v in natural layout with a ones column: [j, head, blk, 65]
        v_ones = singles.tile([128, HPC, NB, D + 1], BF, tag="vo")
        nc.vector.memset(v_ones[:, :, :, D:D + 1], 1.0)

        # ---- projections ----
        copy_ctr = [0]

        def psum_copy(dst, src):
            # alternate projection-copy engine to spread the load
            if copy_ctr[0] % 2 == 0:
                nc.vector.tensor_copy(dst, src)
            else:
                nc.scalar.copy(dst, src)
            copy_ctr[0] += 1

        def emit_pair(name, src_t, pair_tiles, pair):
            pair_sb = pair_tiles[pair]
            steps = []
            for q4 in range(4):
                def step(q4=q4):
                    ps = spool.tile([128, 512], F32, tag="ps", name="ps")
                    gl = slice(q4 * 512, (q4 + 1) * 512)
                    for cc in range(CCH):
                        nc.tensor.matmul(
                            ps[:],
                            lhsT=w_sb[name][:, cc,
                                            pair * 128:(pair + 1) * 128],
                            rhs=src_t[:, cc, gl],
                            start=(cc == 0), stop=(cc == CCH - 1))
                    psum_copy(pair_sb[:, gl], ps[:])
                steps.append(step)
            return steps

        def emit_v_block(nb):
            pv = spool.tile([128, 512], F32, tag="ps", name="ps")
            for cc in range(CCH):
                nc.tensor.matmul(
                    pv[:, 0:DHC],
                    lhsT=xt_sb[:, cc, nb * 128:(nb + 1) * 128],
                    rhs=w_sb["wv"][:, cc, :],
                    start=(cc == 0), stop=(cc == CCH - 1))
            psum_copy(
                v_ones[:, :, nb, 0:D],
                pv[:, 0:DHC].rearrange("p (h d) -> p h d", h=HPC))

        ksteps = emit_pair("wk", xt_sb, kt_p, 0)
        asteps = emit_pair("wa", x2t_sb, at_p, 0)
        for i in range(4):
            ksteps[i]()
            asteps[i]()
            for nb in range(4 * i, 4 * i + 4):
                emit_v_block(nb)

        # filler queues per phase-pair index (0: p0A1, 1: p0A2, 2: p1A1)
        fillers = {
            0: emit_pair("wq", xt_sb, qt_p, 0),
            1: emit_pair("wk", xt_sb, kt_p, 1)
            + emit_pair("wa", x2t_sb, at_p, 1),
            2: emit_pair("wq", xt_sb, qt_p, 1),
        }

        # ---- attention ----
        exp_ctr = [0]

        def emit_exp(pt_ap, ps_ap):
            if (exp_ctr[0] % 16) < ACT_OF_16:
                nc.scalar.activation(pt_ap, ps_ap, EXP, scale=SCALE)
            else:
                nc.vector.tensor_scalar(
                    pt_ap.bitcast(I16), ps_ap, EXP_K, EXP_B, MULT, ADD)
            exp_ctr[0] += 1

        def blk_off(k):  # 8 blocks per pass in a [128,1024] 2-bank acc
            return 65 * k if k < 7 else 512

        for pair in range(2):
            tmp_h = None
            for stage in (1, 2):
                if stage == 1:
                    lhs_t, rhs_t = kt_p[pair], at_p[pair]
                    tmp_h = [
                        tmpo_pool.tile([128, NB, D + 1], BF, tag=f"to{hh}",
                                       name=f"to{hh}")
                        for hh in range(2)]
                    for hh in range(2):
                        nc.vector.memset(tmp_h[hh][:, :, D:D + 1], 1.0)
                    av_rhs = [v_ones[:, 2 * pair + hh] for hh in range(2)]
                else:
                    lhs_t, rhs_t = at_p[pair], qt_p[pair]
                    av_rhs = [tmp_h[hh] for hh in range(2)]
                fill = fillers.get(pair * 2 + (stage - 1), [])

                for pss in range(2):
                    accs = [apool.tile([128, 1024], F32, tag=f"acc{hh}",
                                       name=f"acc{hh}") for hh in range(2)]

                    def do_av(item):
                        j, ch, pts = item
                        for hh in range(2):
                            rhs = av_rhs[hh][:, j, :]
                            for s in range(4):
                                kblk = ch * 4 + s
                                off = blk_off(kblk)
                                nc.tensor.matmul(
                                    accs[hh][:, off:off + D + 1],
                                    lhsT=pts[hh][:, s * 128:(s + 1) * 128],
                                    rhs=rhs,
                                    start=(j == 0 and kblk in (0, 7)),
                                    stop=(j == NB - 1),
                                    skip_group_check=True)

                    pend = []
                    for j in range(NB):
                        for ch in range(2):
                            mwin = slice(pss * 1024 + ch * 512,
                                         pss * 1024 + (ch + 1) * 512)
                            pts = []
                            pss_tiles = []
                            for hh in range(2):
                                base = hh * 64
                                ps = spool.tile([128, 512], F32, tag="ps",
                                                name="ps")
                                nc.tensor.matmul(
                                    ps[:],
                                    lhsT=lhs_t[base:base + 64,
                                               j * 128:(j + 1) * 128],
                                    rhs=rhs_t[base:base + 64, mwin],
                                    start=True, stop=True)
                                pss_tiles.append(ps)
                            for hh in range(2):
                                pt = ppool.tile([128, 512], BF, tag="pt",
                                                name="pt")
                                emit_exp(pt[:], pss_tiles[hh][:])
                                pts.append(pt)
                            if fill:
                                fill.pop(0)()
                            pend.append((j, ch, pts))
                            if len(pend) > LAG:
                                do_av(pend.pop(0))
                    while pend:
                        do_av(pend.pop(0))

                    # ---- pass tails ----
                    for hh in range(2):
                        acc = accs[hh]
                        rec = recp.tile([128, 8], F32, tag="rec", name="rec")
                        v7 = acc[:, 0:455].rearrange("p (k c) -> p k c",
                                                     c=D + 1)
                        nc.vector.reciprocal(rec[:, 0:7], v7[:, :, D])
                        nc.vector.reciprocal(rec[:, 7:8],
                                             acc[:, 512 + D:512 + D + 1])
                        if stage == 1:
                            dst7 = tmp_h[hh][:, pss * 8:pss * 8 + 7, 0:D]
                            dstl = tmp_h[hh][:, pss * 8 + 7:pss * 8 + 8, 0:D]
                        else:
                            hg = 2 * pair + hh
                            dst7 = ot_all[:, pss * 8:pss * 8 + 7, hg, :]
                            dstl = ot_all[:, pss * 8 + 7:pss * 8 + 8, hg, :]
                        nc.vector.tensor_tensor(
                            dst7, v7[:, :, 0:D],
                            rec[:, 0:7, None].to_broadcast((128, 7, D)),
                            MULT)
                        nc.vector.tensor_tensor(
                            dstl,
                            acc[:, 512:512 + D].rearrange(
                                "p (k c) -> p k c", k=1),
                            rec[:, 7:8, None].to_broadcast((128, 1, D)),
                            MULT)
                while fill:
                    fill.pop(0)()

        # ---- output DMA ----
        out_r = out_d.rearrange("(b p) c -> p b c", p=128)
        nc.sync.dma_start(out=out_r[:, 0:NB // 2, :],
                          in_=ot_all[:, 0:NB // 2])
        nc.scalar.dma_start(out=out_r[:, NB // 2:NB, :],
                            in_=ot_all[:, NB // 2:NB])


def _get_program(nreps=1):
    key = f"nc{nreps}"
    if key not in _CACHE:
        _CACHE[key] = _build_program(nreps)
    return _CACHE[key]


def _prep_inputs(x, x2, Wq, Wa, Wkv):
    """Host-side shard prep: transpose + cast to bf16 once per batch/group."""
    xt = [np.ascontiguousarray(x[b].T).astype(BF16) for b in range(2)]
    x2t = [np.ascontiguousarray(x2[b].T).astype(BF16) for b in range(2)]
    wq_t = np.ascontiguousarray(Wq.T).astype(BF16)     # [C, C]
    wa_t = np.ascontiguousarray(Wa.T).astype(BF16)
    wkv_t = np.ascontiguousarray(Wkv.T).astype(BF16)   # [C, 2C]
    in_maps = []
    for i in range(N_CORES):
        b, g = divmod(i, HPC)
        cols = slice(g * DHC, (g + 1) * DHC)
        in_maps.append({
            "xt": xt[b],
            "x2t": x2t[b],
            "wq": np.ascontiguousarray(wq_t[:, cols]),
            "wa": np.ascontiguousarray(wa_t[:, cols]),
            "wk": np.ascontiguousarray(wkv_t[:, cols]),
            "wv": np.ascontiguousarray(
                wkv_t[:, C + g * DHC: C + (g + 1) * DHC]),
        })
    return in_maps


def kernel(x, x2, Wq, Wa, Wkv, _trace=False, _trace_kwargs=None, _nreps=1):
    nc = _get_program(_nreps)
    in_maps = _prep_inputs(
        np.asarray(x, np.float32), np.asarray(x2, np.float32),
        np.asarray(Wq, np.float32), np.asarray(Wa, np.float32),
        np.asarray(Wkv, np.float32))
    res = run_bass_kernel_spmd(nc, in_maps, list(range(N_CORES)),
                               trace=_trace, **(_trace_kwargs or {}))
    out = np.empty((2, N, C), np.float32)
    for i in range(N_CORES):
        b, g = divmod(i, HPC)
        out[b][:, g * DHC:(g + 1) * DHC] = np.asarray(res.results[i]["out"],
                                                      np.float32)
    if _trace:
        return out, res
    return out
